# revision 32
# baseline (speedup 1.0000x reference)
"""Trainium2 Bass kernel for nn_Block_self_attention_inter_intra_3D.

Math: the reference loops 36 overlapping windows (i,j in 0..2, z in 0..3) of a
(2,64,48,48,16) volume, runs channel-projected position attention inside each
window (reading the ORIGINAL x), and writes results back last-write-wins, so
each window "owns" exactly its local [0:16,0:16,0:4] sub-box (1024 queries)
attending to all N window key positions (N in {3456,2304,1536,1024}).

Device math per (window,batch) task, with everything foldable precomputed on
the host:
  energy[m,n] = qag[:,n] . xk[:,m]     qag = (Wk^T Wq) xq + Wk^T bq  (host),
                                       xk = raw window channels (64, NK).
                                       The k-bias term bk contributes a
                                       per-query constant => softmax-invariant
                                       => dropped.  K = 64 on the PE.
  w = exp(energy)                      split across TWO engines (see below)
  outT[n, 0:64] += w[m,n] * vT[m, :]   vT = (Wv x + bv)^T (host, bf16), with
  outT[n, 64]   += w[m,n]              an appended ones column for sum(exp).
  host: out = outT[:,:64]/outT[:,64:] + xq   (normalize + residual on host)

The att@v matmul runs TRANSPOSED (queries on PSUM partitions, 65 channels
free, bf16 operands): 8 x 65 = 520 PE rows per 128-key tile instead of 1024,
cutting PE time ~2x for that stage and making sum(exp) a PSUM column.

exp is the classic wall (ScalarE activation = 1 col/cycle @1.2GHz).  Tiles of
energies alternate between TWO engines (pattern 8:7 per 15 m-tiles):
  S: ScalarE true exp -> bf16
  D: DVE Schraudolph exp: bits16 = round(E*(2^7/ln2) + (127*2^7 - 7.75)) as a
     single fused tensor_scalar (mult+add, f32 PSUM -> int16 SBUF), bitcast
     to bf16.  HW-verified round-to-nearest conversion; max ~3% sawtooth
     error on those tiles only; end-to-end rel err ~3e-3 (tol 2e-2).
Both engines run concurrently on different PSUM energy tiles (3-deep ring of
(128,1024) tiles = 6 banks; att@v accumulator (128,1024) = 2 banks, chunks
packed 4-per-bank at 65-col offsets so no matmul crosses a bank).

Sharding: 72 (window,batch) tasks -> 8 cores x 9 uniform slots
(3x N=3456, 4x N=2304, 2x N=1536; the two N=1024 tasks land on core 7 padded
to 1536 with zeroed keys AND zeroed vT columns -- zero vT (incl. the ones
col) makes fake keys contribute exactly nothing, no mask needed).
"""

import os
import sys

sys.path.insert(0, "/opt/trn_rl_repo")

from contextlib import ExitStack

import numpy as np
import ml_dtypes

import concourse.bacc as bacc
import concourse.mybir as mybir
import concourse.tile as tile
from concourse.bass_utils import run_bass_kernel_spmd

F32 = mybir.dt.float32
F32R = mybir.dt.float32r
BF16 = mybir.dt.bfloat16
I16 = mybir.dt.int16
BF = ml_dtypes.bfloat16

N_CORES = 8
NQ = 1024
SLOT_NK = [3456, 3456, 3456, 2304, 2304, 2304, 2304, 1536, 1536]

# Schraudolph constants for bf16-bit exp on the DVE (round-to-nearest HW
# conversion, calibrated end-to-end: c = 7.75)
SCH_A = float(np.float32(128.0 / np.log(2.0)))
SCH_B = float(np.float32(127.0 * 128.0 - 7.75))

B, C, H, W, T = 2, 64, 48, 48, 16


def _win(i):
    s = 16 * i
    return s, min(s + 24, 48) - s


def _win_z(z):
    s = 4 * z
    return s, min(s + 6, 16) - s


def _task_lists():
    t3456 = [(b, i, j, z) for b in (0, 1) for i in (0, 1) for j in (0, 1)
             for z in (0, 1, 2)]
    t2304 = ([(b, i, j, 3) for b in (0, 1) for i in (0, 1) for j in (0, 1)] +
             [(b, i, 2, z) for b in (0, 1) for i in (0, 1) for z in (0, 1, 2)] +
             [(b, 2, j, z) for b in (0, 1) for j in (0, 1) for z in (0, 1, 2)])
    t1536 = ([(b, i, 2, 3) for b in (0, 1) for i in (0, 1)] +
             [(b, 2, j, 3) for b in (0, 1) for j in (0, 1)] +
             [(b, 2, 2, z) for b in (0, 1) for z in (0, 1, 2)])
    t1024 = [(b, 2, 2, 3) for b in (0, 1)]
    assert len(t3456) == 24 and len(t2304) == 32
    assert len(t1536) == 14 and len(t1024) == 2
    tail = t1536 + t1024
    per_core = []
    for c in range(N_CORES):
        per_core.append(t3456[3 * c:3 * c + 3] + t2304[4 * c:4 * c + 4] +
                        tail[2 * c:2 * c + 2])
    return per_core


TASKS = _task_lists()

# smallest slot first so the pipeline fills during the first (small) DMAs
ORDER1 = [7, 0, 3, 1, 4, 2, 5, 6, 8]


def _sd_is_scalar(g):
    """Exp engine for global m-tile g: 8 ScalarE / 7 DVE per 15 tiles."""
    return (g % 15) % 2 == 0


def _emit(nc, tc, ctx, aps, reps):
    sbk = ctx.enter_context(tc.tile_pool(name="sbk", bufs=3))
    sbq = ctx.enter_context(tc.tile_pool(name="sbq", bufs=3))
    sbv = ctx.enter_context(tc.tile_pool(name="sbv", bufs=3))
    expp = ctx.enter_context(tc.tile_pool(name="expp", bufs=4))
    sbo = ctx.enter_context(tc.tile_pool(name="sbo", bufs=2))
    # PSUM: pse 3 x (128,1024) = 6 banks, pso 1 x (128,1024) = 2 banks
    pse = ctx.enter_context(tc.tile_pool(name="pse", bufs=3, space="PSUM"))
    pso = ctx.enter_context(tc.tile_pool(name="pso", bufs=1, space="PSUM"))

    Exp = mybir.ActivationFunctionType.Exp
    Mult = mybir.AluOpType.mult
    Add = mybir.AluOpType.add

    # PE p-state warm-up on memset garbage: the ramp completes during the
    # first input DMA waits instead of slowing the first energy matmuls
    ws = sbk.tile([64, 640], F32, tag="wsrc")
    nc.gpsimd.memset(ws[:], 0.01)
    wps = pse.tile([128, 1024], F32, tag="e")
    for _ in range(5):
        nc.tensor.matmul(wps[:, 0:512], ws[:, 0:128].bitcast(F32R),
                         ws[:, 128:640].bitcast(F32R), start=True, stop=True)

    order = ORDER1 * reps
    n = len(order)

    def prologue(idx, chunked=False):
        s = order[idx]
        nk = SLOT_NK[s]
        mt = nk // 128
        xk = sbk.tile([64, nk], F32R, tag="xk")
        qag = sbq.tile([64, NQ], F32R, tag="qag")
        vt = sbv.tile([128, mt * 65], BF16, tag="vt")
        if chunked:
            # issue the first energy tile's operands first: one xk m-tile
            # (tiny transfer) and the first qag half
            nc.sync.dma_start(xk[:, 0:128], aps[f"xk{s}"][:, 0:128])
            nc.sync.dma_start(qag[:, 0:512], aps[f"qag{s}"][:, 0:512])
            nc.sync.dma_start(qag[:, 512:1024], aps[f"qag{s}"][:, 512:1024])
            off = 128
            while off < nk:
                w = min(512, nk - off)
                nc.sync.dma_start(xk[:, off:off + w],
                                  aps[f"xk{s}"][:, off:off + w])
                off += w
        else:
            nc.sync.dma_start(qag[:], aps[f"qag{s}"][:])
            nc.sync.dma_start(xk[:], aps[f"xk{s}"][:])
        nc.sync.dma_start(vt[:], aps[f"vt{s}"][:])
        vt3 = vt[:].rearrange("p (t c) -> p t c", c=65)
        return dict(s=s, nk=nk, mt=mt, xk=xk, qag=qag, vt3=vt3)

    tiles = []
    for idx in range(n):
        mt = SLOT_NK[order[idx]] // 128
        tiles += [(idx, t, mt) for t in range(mt)]
    NT = len(tiles)

    sts = {0: prologue(0, chunked=True)}
    if n > 1:
        sts[1] = prologue(1)

    def energy(g):
        idx, t, _ = tiles[g]
        st = sts[idx]
        eps = pse.tile([128, 1024], F32, tag="e")
        with tc.high_priority(offset=100000):
            for o in (0, 512):
                nc.tensor.matmul(eps[:, o:o + 512],
                                 st["xk"][:, 128 * t:128 * t + 128],
                                 st["qag"][:, o:o + 512],
                                 start=True, stop=True)
        return eps

    def expop(g, eps):
        if _sd_is_scalar(g):
            ex = expp.tile([128, 1024], BF16, tag="exs")
            nc.scalar.activation(ex[:], eps[:], Exp)
            return (ex, False)
        ex = expp.tile([128, 1024], I16, tag="exd")
        nc.vector.tensor_scalar(ex[:], eps[:], SCH_A, SCH_B, Mult, Add)
        return (ex, True)

    def attv(g, ops3, exinfo):
        idx, t, mt = tiles[g]
        st = sts[idx]
        ex, isd = exinfo
        for c in range(8):
            lhsT = ex[:, 128 * c:128 * c + 128]
            if isd:
                lhsT = lhsT.bitcast(BF16)
            co = (c % 4) * 65
            # start=True pending-zeroes the ENTIRE 2KB PSUM bank, so only
            # the first chunk of each bank may set it; the other chunks'
            # first write then lands on pending-zero bytes and overwrites
            # (zero+accumulate) correctly.
            nc.tensor.matmul(ops3[:, c // 4, co:co + 65], lhsT,
                             st["vt3"][:, t, :],
                             start=(t == 0 and c % 4 == 0),
                             stop=(t == mt - 1 and c % 4 == 3),
                             skip_group_check=True)

    def boundary(idx, ops, blocking_exp_scalar, last=False):
        """PSUM evacuation.  The next slot's first att@v waits on BOTH the
        next tile's exp (engine Y) and these copies, so the copies go on the
        OTHER engine Z to run concurrently with that exp.  Per-bank split
        releases the two pso banks' WARs independently."""
        st = sts[idx]
        fin = sbo.tile([128, 520], F32, tag="fin")
        src = ops[:].rearrange("p (g x) -> p g x", x=512)
        nc.scalar.copy(fin[:, 0:260], src[:, 0, 0:260])
        nc.sync.dma_start(aps["o"][st["s"]][:, 0:260], fin[:, 0:260])
        nc.vector.tensor_copy(fin[:, 260:520], src[:, 1, 0:260])
        nc.sync.dma_start(aps["o"][st["s"]][:, 260:520], fin[:, 260:520])

    # att@v lags one tile behind the emission front: the PE stream becomes
    # [E(g+2)ab, A(g-1)x8] so exp leads attv by 2 tiles (jitter absorption)
    # and a full energy pair sits between consecutive slots' attv groups,
    # covering the boundary-copy latency.
    epst = {0: energy(0)}
    if NT > 1:
        epst[1] = energy(1)
    exinfo = {0: expop(0, epst[0])}
    opst = {}
    for g in range(NT + 1):
        if g < NT:
            idx, t, mt = tiles[g]
            if t == 0 and idx + 2 < n:
                sts[idx + 2] = prologue(idx + 2)
            if g + 2 < NT:
                epst[g + 2] = energy(g + 2)
            if g + 1 < NT:
                exinfo[g + 1] = expop(g + 1, epst[g + 1])
            if t == 0:
                ops_t = pso.tile([128, 1024], F32, tag="o")
                opst[idx] = ops_t
        if g >= 1:
            pidx, pt, pmt = tiles[g - 1]
            ops3 = opst[pidx][:].rearrange("p (g x) -> p g x", x=512)
            attv(g - 1, ops3, exinfo.pop(g - 1))
            epst.pop(g - 1)
            if pt == pmt - 1:
                boundary(pidx, opst.pop(pidx), True, last=(g - 1 == NT - 1))
                sts.pop(pidx)


_CACHE = {}


def _build(reps):
    if reps in _CACHE:
        return _CACHE[reps]
    nc = bacc.Bacc("TRN2", target_bir_lowering=False, debug=False,
                   enable_asserts=True)
    aps = {}
    for s, nk in enumerate(SLOT_NK):
        aps[f"xk{s}"] = nc.dram_tensor(f"xk{s}", [64, nk], F32R,
                                       kind="ExternalInput").ap()
        aps[f"qag{s}"] = nc.dram_tensor(f"qag{s}", [64, NQ], F32R,
                                        kind="ExternalInput").ap()
        aps[f"vt{s}"] = nc.dram_tensor(f"vt{s}", [128, (nk // 128) * 65],
                                       BF16, kind="ExternalInput").ap()
    aps["o"] = nc.dram_tensor("o", [9, 128, 520], F32,
                              kind="ExternalOutput").ap()

    with tile.TileContext(nc) as tc:
        with ExitStack() as ctx:
            _emit(nc, tc, ctx, aps, reps)
    nc.compile()
    _CACHE[reps] = nc
    return nc


def _host_inputs(x, Wq, bq, Wk, bk, Wv, bv):
    x = np.asarray(x, np.float32)
    Wq = np.asarray(Wq, np.float32)
    Wk = np.asarray(Wk, np.float32)
    Wv = np.asarray(Wv, np.float32)
    bq = np.asarray(bq, np.float32)
    bv = np.asarray(bv, np.float32)

    xf = x.reshape(B, C, -1)
    Aq = Wk.T @ Wq                      # = (Wq^T Wk)^T
    cvec = Wk.T @ bq
    qag_full = (Aq @ xf + cvec[None, :, None]).reshape(B, C, H, W, T)
    v_full = (Wv @ xf + bv[None, :, None]).reshape(B, C, H, W, T)

    in_maps = []
    for c in range(N_CORES):
        m = {}
        for s, (b, i, j, z) in enumerate(TASKS[c]):
            nk_slot = SLOT_NK[s]
            mt = nk_slot // 128
            sx, dx = _win(i)
            sy, dy = _win(j)
            sz, dz = _win_z(z)
            nk = dx * dy * dz
            xkb = np.zeros((64, nk_slot), np.float32)
            xkb[:, :nk] = x[b, :, sx:sx + dx, sy:sy + dy,
                            sz:sz + dz].reshape(64, nk)
            m[f"xk{s}"] = xkb
            m[f"qag{s}"] = np.ascontiguousarray(
                qag_full[b, :, sx:sx + 16, sy:sy + 16,
                         sz:sz + 4].reshape(64, NQ))
            vp = np.zeros((65, nk_slot), np.float32)
            vp[:64, :nk] = v_full[b, :, sx:sx + dx, sy:sy + dy,
                                  sz:sz + dz].reshape(64, nk)
            vp[64, :nk] = 1.0
            m[f"vt{s}"] = np.ascontiguousarray(
                vp.reshape(65, mt, 128).transpose(2, 1, 0)
                .reshape(128, mt * 65)).astype(BF)
        in_maps.append(m)
    return in_maps


def _scatter(results, x):
    x = np.asarray(x, np.float32)
    out = np.empty((B, C, H, W, T), np.float32)
    for c in range(N_CORES):
        o = results[c]["o"]
        for s, (b, i, j, z) in enumerate(TASKS[c]):
            sx, _ = _win(i)
            sy, _ = _win(j)
            sz, _ = _win_z(z)
            fin3 = o[s].reshape(128, 2, 260)
            outT = np.empty((1024, 65), np.float32)
            for ch in range(8):
                outT[128 * ch:128 * ch + 128] = (
                    fin3[:, ch // 4, (ch % 4) * 65:(ch % 4) * 65 + 65])
            onrm = outT[:, :64] / outT[:, 64:65]
            blk = (onrm.T.reshape(64, 16, 16, 4) +
                   x[b, :, sx:sx + 16, sy:sy + 16, sz:sz + 4])
            out[b, :, sx:sx + 16, sy:sy + 16, sz:sz + 4] = blk
    return out


def _ensure_axon():
    # The axon PJRT plugin is registered by sitecustomize at interpreter
    # start; if a caller pinned JAX_PLATFORMS=cpu before jax init, try to
    # re-enable the axon backend (run_bass_via_pjrt needs 8 trn2 devices).
    import jax

    try:
        if any(d.platform == "axon" for d in jax.devices()):
            return
    except Exception:
        pass
    try:
        jax.config.update("jax_platforms", "axon,cpu")
        jax.extend.backend.clear_backends()
    except Exception:
        pass


def run(x, Wq, bq, Wk, bk, Wv, bv, reps=1):
    _ensure_axon()
    nc = _build(reps)
    in_maps = _host_inputs(x, Wq, bq, Wk, bk, Wv, bv)
    res = run_bass_kernel_spmd(nc, in_maps, core_ids=list(range(N_CORES)))
    return _scatter(res.results, x), res


def kernel(x, Wq, bq, Wk, bk, Wv, bv):
    out, _ = run(x, Wq, bq, Wk, bk, Wv, bv,
                 reps=int(os.environ.get("KREP", "1")))
    return out


# revision 34
# speedup vs baseline: 1.0004x; 1.0004x over previous
"""Trainium2 Bass kernel for nn_Block_self_attention_inter_intra_3D.

Math: the reference loops 36 overlapping windows (i,j in 0..2, z in 0..3) of a
(2,64,48,48,16) volume, runs channel-projected position attention inside each
window (reading the ORIGINAL x), and writes results back last-write-wins, so
each window "owns" exactly its local [0:16,0:16,0:4] sub-box (1024 queries)
attending to all N window key positions (N in {3456,2304,1536,1024}).

Device math per (window,batch) task, with everything foldable precomputed on
the host:
  energy[m,n] = qag[:,n] . xk[:,m]     qag = (Wk^T Wq) xq + Wk^T bq  (host),
                                       xk = raw window channels (64, NK).
                                       The k-bias term bk contributes a
                                       per-query constant => softmax-invariant
                                       => dropped.  K = 64 on the PE.
  w = exp(energy)                      split across TWO engines (see below)
  outT[n, 0:64] += w[m,n] * vT[m, :]   vT = (Wv x + bv)^T (host, bf16), with
  outT[n, 64]   += w[m,n]              an appended ones column for sum(exp).
  host: out = outT[:,:64]/outT[:,64:] + xq   (normalize + residual on host)

The att@v matmul runs TRANSPOSED (queries on PSUM partitions, 65 channels
free, bf16 operands): 8 x 65 = 520 PE rows per 128-key tile instead of 1024,
cutting PE time ~2x for that stage and making sum(exp) a PSUM column.

exp is the classic wall (ScalarE activation = 1 col/cycle @1.2GHz).  Tiles of
energies alternate between TWO engines (pattern 8:7 per 15 m-tiles):
  S: ScalarE true exp -> bf16
  D: DVE Schraudolph exp: bits16 = round(E*(2^7/ln2) + (127*2^7 - 7.75)) as a
     single fused tensor_scalar (mult+add, f32 PSUM -> int16 SBUF), bitcast
     to bf16.  HW-verified round-to-nearest conversion; max ~3% sawtooth
     error on those tiles only; end-to-end rel err ~3e-3 (tol 2e-2).
Both engines run concurrently on different PSUM energy tiles (3-deep ring of
(128,1024) tiles = 6 banks; att@v accumulator (128,1024) = 2 banks, chunks
packed 4-per-bank at 65-col offsets so no matmul crosses a bank).

Sharding: 72 (window,batch) tasks -> 8 cores x 9 uniform slots
(3x N=3456, 4x N=2304, 2x N=1536; the two N=1024 tasks land on core 7 padded
to 1536 with zeroed keys AND zeroed vT columns -- zero vT (incl. the ones
col) makes fake keys contribute exactly nothing, no mask needed).
"""

import os
import sys

sys.path.insert(0, "/opt/trn_rl_repo")

from contextlib import ExitStack

import numpy as np
import ml_dtypes

import concourse.bacc as bacc
import concourse.mybir as mybir
import concourse.tile as tile
from concourse.bass_utils import run_bass_kernel_spmd

F32 = mybir.dt.float32
F32R = mybir.dt.float32r
BF16 = mybir.dt.bfloat16
I16 = mybir.dt.int16
BF = ml_dtypes.bfloat16

N_CORES = 8
NQ = 1024
SLOT_NK = [3456, 3456, 3456, 2304, 2304, 2304, 2304, 1536, 1536]

# Schraudolph constants for bf16-bit exp on the DVE (round-to-nearest HW
# conversion, calibrated end-to-end: c = 7.75)
SCH_A = float(np.float32(128.0 / np.log(2.0)))
SCH_B = float(np.float32(127.0 * 128.0 - 7.75))

B, C, H, W, T = 2, 64, 48, 48, 16


def _win(i):
    s = 16 * i
    return s, min(s + 24, 48) - s


def _win_z(z):
    s = 4 * z
    return s, min(s + 6, 16) - s


def _task_lists():
    t3456 = [(b, i, j, z) for b in (0, 1) for i in (0, 1) for j in (0, 1)
             for z in (0, 1, 2)]
    t2304 = ([(b, i, j, 3) for b in (0, 1) for i in (0, 1) for j in (0, 1)] +
             [(b, i, 2, z) for b in (0, 1) for i in (0, 1) for z in (0, 1, 2)] +
             [(b, 2, j, z) for b in (0, 1) for j in (0, 1) for z in (0, 1, 2)])
    t1536 = ([(b, i, 2, 3) for b in (0, 1) for i in (0, 1)] +
             [(b, 2, j, 3) for b in (0, 1) for j in (0, 1)] +
             [(b, 2, 2, z) for b in (0, 1) for z in (0, 1, 2)])
    t1024 = [(b, 2, 2, 3) for b in (0, 1)]
    assert len(t3456) == 24 and len(t2304) == 32
    assert len(t1536) == 14 and len(t1024) == 2
    tail = t1536 + t1024
    per_core = []
    for c in range(N_CORES):
        per_core.append(t3456[3 * c:3 * c + 3] + t2304[4 * c:4 * c + 4] +
                        tail[2 * c:2 * c + 2])
    return per_core


TASKS = _task_lists()

# smallest slot first so the pipeline fills during the first (small) DMAs
ORDER1 = [7, 0, 3, 1, 4, 2, 5, 6, 8]


def _sd_is_scalar(g):
    """Exp engine for global m-tile g: 8 ScalarE / 7 DVE per 15 tiles."""
    return (g % 15) % 2 == 0


def _emit(nc, tc, ctx, aps, reps):
    sbk = ctx.enter_context(tc.tile_pool(name="sbk", bufs=3))
    sbq = ctx.enter_context(tc.tile_pool(name="sbq", bufs=3))
    sbv = ctx.enter_context(tc.tile_pool(name="sbv", bufs=3))
    expp = ctx.enter_context(tc.tile_pool(name="expp", bufs=4))
    sbo = ctx.enter_context(tc.tile_pool(name="sbo", bufs=2))
    # PSUM: pse 3 x (128,1024) = 6 banks, pso 1 x (128,1024) = 2 banks
    pse = ctx.enter_context(tc.tile_pool(name="pse", bufs=3, space="PSUM"))
    pso = ctx.enter_context(tc.tile_pool(name="pso", bufs=1, space="PSUM"))

    Exp = mybir.ActivationFunctionType.Exp
    Mult = mybir.AluOpType.mult
    Add = mybir.AluOpType.add

    # PE p-state warm-up on memset garbage: the ramp completes during the
    # first input DMA waits instead of slowing the first energy matmuls
    ws = sbk.tile([64, 640], F32, tag="wsrc")
    nc.vector.memset(ws[:], 0.01)
    wps = pse.tile([128, 1024], F32, tag="e")
    for _ in range(5):
        nc.tensor.matmul(wps[:, 0:512], ws[:, 0:128].bitcast(F32R),
                         ws[:, 128:640].bitcast(F32R), start=True, stop=True)

    order = ORDER1 * reps
    n = len(order)

    def prologue(idx, chunked=False):
        s = order[idx]
        nk = SLOT_NK[s]
        mt = nk // 128
        xk = sbk.tile([64, nk], F32R, tag="xk")
        qag = sbq.tile([64, NQ], F32R, tag="qag")
        vt = sbv.tile([128, mt * 65], BF16, tag="vt")
        if chunked:
            # first energy tile's operands first; the bigger qag half leads
            # because SP dispatches serialize at ~565ns each, which adds
            # directly to the start of every later transfer
            nc.sync.dma_start(qag[:, 0:512], aps[f"qag{s}"][:, 0:512])
            nc.sync.dma_start(xk[:, 0:128], aps[f"xk{s}"][:, 0:128])
            nc.sync.dma_start(qag[:, 512:1024], aps[f"qag{s}"][:, 512:1024])
            off = 128
            while off < nk:
                w = min(512, nk - off)
                nc.sync.dma_start(xk[:, off:off + w],
                                  aps[f"xk{s}"][:, off:off + w])
                off += w
        else:
            nc.sync.dma_start(qag[:], aps[f"qag{s}"][:])
            nc.sync.dma_start(xk[:], aps[f"xk{s}"][:])
        nc.sync.dma_start(vt[:], aps[f"vt{s}"][:])
        vt3 = vt[:].rearrange("p (t c) -> p t c", c=65)
        return dict(s=s, nk=nk, mt=mt, xk=xk, qag=qag, vt3=vt3)

    tiles = []
    for idx in range(n):
        mt = SLOT_NK[order[idx]] // 128
        tiles += [(idx, t, mt) for t in range(mt)]
    NT = len(tiles)

    sts = {0: prologue(0, chunked=True)}
    if n > 1:
        sts[1] = prologue(1)

    def energy(g):
        idx, t, _ = tiles[g]
        st = sts[idx]
        eps = pse.tile([128, 1024], F32, tag="e")
        with tc.high_priority(offset=100000):
            for o in (0, 512):
                nc.tensor.matmul(eps[:, o:o + 512],
                                 st["xk"][:, 128 * t:128 * t + 128],
                                 st["qag"][:, o:o + 512],
                                 start=True, stop=True)
        return eps

    def expop(g, eps):
        if _sd_is_scalar(g):
            ex = expp.tile([128, 1024], BF16, tag="exs")
            nc.scalar.activation(ex[:], eps[:], Exp)
            return (ex, False)
        ex = expp.tile([128, 1024], I16, tag="exd")
        nc.vector.tensor_scalar(ex[:], eps[:], SCH_A, SCH_B, Mult, Add)
        return (ex, True)

    def attv(g, ops3, exinfo):
        idx, t, mt = tiles[g]
        st = sts[idx]
        ex, isd = exinfo
        for c in range(8):
            lhsT = ex[:, 128 * c:128 * c + 128]
            if isd:
                lhsT = lhsT.bitcast(BF16)
            co = (c % 4) * 65
            # start=True pending-zeroes the ENTIRE 2KB PSUM bank, so only
            # the first chunk of each bank may set it; the other chunks'
            # first write then lands on pending-zero bytes and overwrites
            # (zero+accumulate) correctly.
            nc.tensor.matmul(ops3[:, c // 4, co:co + 65], lhsT,
                             st["vt3"][:, t, :],
                             start=(t == 0 and c % 4 == 0),
                             stop=(t == mt - 1 and c % 4 == 3),
                             skip_group_check=True)

    def boundary(idx, ops, blocking_exp_scalar, last=False):
        """PSUM evacuation.  The next slot's first att@v waits on BOTH the
        next tile's exp (engine Y) and these copies, so the copies go on the
        OTHER engine Z to run concurrently with that exp.  Per-bank split
        releases the two pso banks' WARs independently."""
        st = sts[idx]
        fin = sbo.tile([128, 520], F32, tag="fin")
        src = ops[:].rearrange("p (g x) -> p g x", x=512)
        nc.scalar.copy(fin[:, 0:260], src[:, 0, 0:260])
        nc.sync.dma_start(aps["o"][st["s"]][:, 0:260], fin[:, 0:260])
        nc.vector.tensor_copy(fin[:, 260:520], src[:, 1, 0:260])
        nc.sync.dma_start(aps["o"][st["s"]][:, 260:520], fin[:, 260:520])

    # att@v lags one tile behind the emission front: the PE stream becomes
    # [E(g+2)ab, A(g-1)x8] so exp leads attv by 2 tiles (jitter absorption)
    # and a full energy pair sits between consecutive slots' attv groups,
    # covering the boundary-copy latency.
    epst = {0: energy(0)}
    if NT > 1:
        epst[1] = energy(1)
    exinfo = {0: expop(0, epst[0])}
    opst = {}
    for g in range(NT + 1):
        if g < NT:
            idx, t, mt = tiles[g]
            if t == 0 and idx + 2 < n:
                sts[idx + 2] = prologue(idx + 2)
            if g + 2 < NT:
                epst[g + 2] = energy(g + 2)
            if g + 1 < NT:
                exinfo[g + 1] = expop(g + 1, epst[g + 1])
            if t == 0:
                ops_t = pso.tile([128, 1024], F32, tag="o")
                opst[idx] = ops_t
        if g >= 1:
            pidx, pt, pmt = tiles[g - 1]
            ops3 = opst[pidx][:].rearrange("p (g x) -> p g x", x=512)
            attv(g - 1, ops3, exinfo.pop(g - 1))
            epst.pop(g - 1)
            if pt == pmt - 1:
                boundary(pidx, opst.pop(pidx), True, last=(g - 1 == NT - 1))
                sts.pop(pidx)


_CACHE = {}


def _build(reps):
    if reps in _CACHE:
        return _CACHE[reps]
    nc = bacc.Bacc("TRN2", target_bir_lowering=False, debug=False,
                   enable_asserts=True)
    aps = {}
    for s, nk in enumerate(SLOT_NK):
        aps[f"xk{s}"] = nc.dram_tensor(f"xk{s}", [64, nk], F32R,
                                       kind="ExternalInput").ap()
        aps[f"qag{s}"] = nc.dram_tensor(f"qag{s}", [64, NQ], F32R,
                                        kind="ExternalInput").ap()
        aps[f"vt{s}"] = nc.dram_tensor(f"vt{s}", [128, (nk // 128) * 65],
                                       BF16, kind="ExternalInput").ap()
    aps["o"] = nc.dram_tensor("o", [9, 128, 520], F32,
                              kind="ExternalOutput").ap()

    with tile.TileContext(nc) as tc:
        with ExitStack() as ctx:
            _emit(nc, tc, ctx, aps, reps)
    nc.compile()
    _CACHE[reps] = nc
    return nc


def _host_inputs(x, Wq, bq, Wk, bk, Wv, bv):
    x = np.asarray(x, np.float32)
    Wq = np.asarray(Wq, np.float32)
    Wk = np.asarray(Wk, np.float32)
    Wv = np.asarray(Wv, np.float32)
    bq = np.asarray(bq, np.float32)
    bv = np.asarray(bv, np.float32)

    xf = x.reshape(B, C, -1)
    Aq = Wk.T @ Wq                      # = (Wq^T Wk)^T
    cvec = Wk.T @ bq
    qag_full = (Aq @ xf + cvec[None, :, None]).reshape(B, C, H, W, T)
    v_full = (Wv @ xf + bv[None, :, None]).reshape(B, C, H, W, T)

    in_maps = []
    for c in range(N_CORES):
        m = {}
        for s, (b, i, j, z) in enumerate(TASKS[c]):
            nk_slot = SLOT_NK[s]
            mt = nk_slot // 128
            sx, dx = _win(i)
            sy, dy = _win(j)
            sz, dz = _win_z(z)
            nk = dx * dy * dz
            xkb = np.zeros((64, nk_slot), np.float32)
            xkb[:, :nk] = x[b, :, sx:sx + dx, sy:sy + dy,
                            sz:sz + dz].reshape(64, nk)
            m[f"xk{s}"] = xkb
            m[f"qag{s}"] = np.ascontiguousarray(
                qag_full[b, :, sx:sx + 16, sy:sy + 16,
                         sz:sz + 4].reshape(64, NQ))
            vp = np.zeros((65, nk_slot), np.float32)
            vp[:64, :nk] = v_full[b, :, sx:sx + dx, sy:sy + dy,
                                  sz:sz + dz].reshape(64, nk)
            vp[64, :nk] = 1.0
            m[f"vt{s}"] = np.ascontiguousarray(
                vp.reshape(65, mt, 128).transpose(2, 1, 0)
                .reshape(128, mt * 65)).astype(BF)
        in_maps.append(m)
    return in_maps


def _scatter(results, x):
    x = np.asarray(x, np.float32)
    out = np.empty((B, C, H, W, T), np.float32)
    for c in range(N_CORES):
        o = results[c]["o"]
        for s, (b, i, j, z) in enumerate(TASKS[c]):
            sx, _ = _win(i)
            sy, _ = _win(j)
            sz, _ = _win_z(z)
            fin3 = o[s].reshape(128, 2, 260)
            outT = np.empty((1024, 65), np.float32)
            for ch in range(8):
                outT[128 * ch:128 * ch + 128] = (
                    fin3[:, ch // 4, (ch % 4) * 65:(ch % 4) * 65 + 65])
            onrm = outT[:, :64] / outT[:, 64:65]
            blk = (onrm.T.reshape(64, 16, 16, 4) +
                   x[b, :, sx:sx + 16, sy:sy + 16, sz:sz + 4])
            out[b, :, sx:sx + 16, sy:sy + 16, sz:sz + 4] = blk
    return out


def _ensure_axon():
    # The axon PJRT plugin is registered by sitecustomize at interpreter
    # start; if a caller pinned JAX_PLATFORMS=cpu before jax init, try to
    # re-enable the axon backend (run_bass_via_pjrt needs 8 trn2 devices).
    import jax

    try:
        if any(d.platform == "axon" for d in jax.devices()):
            return
    except Exception:
        pass
    try:
        jax.config.update("jax_platforms", "axon,cpu")
        jax.extend.backend.clear_backends()
    except Exception:
        pass


def run(x, Wq, bq, Wk, bk, Wv, bv, reps=1):
    _ensure_axon()
    nc = _build(reps)
    in_maps = _host_inputs(x, Wq, bq, Wk, bk, Wv, bv)
    res = run_bass_kernel_spmd(nc, in_maps, core_ids=list(range(N_CORES)))
    return _scatter(res.results, x), res


def kernel(x, Wq, bq, Wk, bk, Wv, bv):
    out, _ = run(x, Wq, bq, Wk, bk, Wv, bv,
                 reps=int(os.environ.get("KREP", "1")))
    return out


# revision 35
# speedup vs baseline: 1.0306x; 1.0302x over previous
"""Trainium2 Bass kernel for nn_Block_self_attention_inter_intra_3D.

Math: the reference loops 36 overlapping windows (i,j in 0..2, z in 0..3) of a
(2,64,48,48,16) volume, runs channel-projected position attention inside each
window (reading the ORIGINAL x), and writes results back last-write-wins, so
each window "owns" exactly its local [0:16,0:16,0:4] sub-box (1024 queries)
attending to all N window key positions (N in {3456,2304,1536,1024}).

Device math per (window,batch) task, with everything foldable precomputed on
the host:
  energy[m,n] = qag[:,n] . xk[:,m]     qag = (Wk^T Wq) xq + Wk^T bq  (host),
                                       xk = raw window channels (64, NK).
                                       The k-bias term bk contributes a
                                       per-query constant => softmax-invariant
                                       => dropped.  K = 64 on the PE.
  w = exp(energy)                      split across TWO engines (see below)
  outT[n, 0:64] += w[m,n] * vT[m, :]   vT = (Wv x + bv)^T (host, bf16), with
  outT[n, 64]   += w[m,n]              an appended ones column for sum(exp).
  host: out = outT[:,:64]/outT[:,64:] + xq   (normalize + residual on host)

The att@v matmul runs TRANSPOSED (queries on PSUM partitions, 65 channels
free, bf16 operands): 8 x 65 = 520 PE rows per 128-key tile instead of 1024,
cutting PE time ~2x for that stage and making sum(exp) a PSUM column.

exp is the classic wall (ScalarE activation = 1 col/cycle @1.2GHz).  Tiles of
energies alternate between TWO engines (pattern 8:7 per 15 m-tiles):
  S: ScalarE true exp -> bf16
  D: DVE Schraudolph exp: bits16 = round(E*(2^7/ln2) + (127*2^7 - 7.75)) as a
     single fused tensor_scalar (mult+add, f32 PSUM -> int16 SBUF), bitcast
     to bf16.  HW-verified round-to-nearest conversion; max ~3% sawtooth
     error on those tiles only; end-to-end rel err ~3e-3 (tol 2e-2).
Both engines run concurrently on different PSUM energy tiles (3-deep ring of
(128,1024) tiles = 6 banks; att@v accumulator (128,1024) = 2 banks, chunks
packed 4-per-bank at 65-col offsets so no matmul crosses a bank).

Sharding: 72 (window,batch) tasks -> 8 cores x 9 uniform slots
(3x N=3456, 4x N=2304, 2x N=1536; the two N=1024 tasks land on core 7 padded
to 1536 with zeroed keys AND zeroed vT columns -- zero vT (incl. the ones
col) makes fake keys contribute exactly nothing, no mask needed).
"""

import os
import sys

sys.path.insert(0, "/opt/trn_rl_repo")

from contextlib import ExitStack

import numpy as np
import ml_dtypes

import concourse.bacc as bacc
import concourse.mybir as mybir
import concourse.tile as tile
from concourse.bass_utils import run_bass_kernel_spmd

F32 = mybir.dt.float32
F32R = mybir.dt.float32r
BF16 = mybir.dt.bfloat16
I16 = mybir.dt.int16
BF = ml_dtypes.bfloat16

N_CORES = 8
NQ = 1024
SLOT_NK = [3456, 3456, 3456, 2304, 2304, 2304, 2304, 1536, 1536]

# Schraudolph constants for bf16-bit exp on the DVE (round-to-nearest HW
# conversion, calibrated end-to-end: c = 7.75)
SCH_A = float(np.float32(128.0 / np.log(2.0)))
SCH_B = float(np.float32(127.0 * 128.0 - 7.75))

B, C, H, W, T = 2, 64, 48, 48, 16


def _win(i):
    s = 16 * i
    return s, min(s + 24, 48) - s


def _win_z(z):
    s = 4 * z
    return s, min(s + 6, 16) - s


def _task_lists():
    t3456 = [(b, i, j, z) for b in (0, 1) for i in (0, 1) for j in (0, 1)
             for z in (0, 1, 2)]
    t2304 = ([(b, i, j, 3) for b in (0, 1) for i in (0, 1) for j in (0, 1)] +
             [(b, i, 2, z) for b in (0, 1) for i in (0, 1) for z in (0, 1, 2)] +
             [(b, 2, j, z) for b in (0, 1) for j in (0, 1) for z in (0, 1, 2)])
    t1536 = ([(b, i, 2, 3) for b in (0, 1) for i in (0, 1)] +
             [(b, 2, j, 3) for b in (0, 1) for j in (0, 1)] +
             [(b, 2, 2, z) for b in (0, 1) for z in (0, 1, 2)])
    t1024 = [(b, 2, 2, 3) for b in (0, 1)]
    assert len(t3456) == 24 and len(t2304) == 32
    assert len(t1536) == 14 and len(t1024) == 2
    tail = t1536 + t1024
    per_core = []
    for c in range(N_CORES):
        per_core.append(t3456[3 * c:3 * c + 3] + t2304[4 * c:4 * c + 4] +
                        tail[2 * c:2 * c + 2])
    return per_core


TASKS = _task_lists()

# smallest slot first so the pipeline fills during the first (small) DMAs
ORDER1 = [7, 0, 3, 1, 4, 2, 5, 6, 8]


def _sd_is_scalar(g):
    """Exp engine for global m-tile g: 8 ScalarE / 7 DVE per 15 tiles."""
    return (g % 15) % 2 == 0


def _emit(nc, tc, ctx, aps, reps):
    sbk = ctx.enter_context(tc.tile_pool(name="sbk", bufs=3))
    sbq = ctx.enter_context(tc.tile_pool(name="sbq", bufs=3))
    sbv = ctx.enter_context(tc.tile_pool(name="sbv", bufs=3))
    expp = ctx.enter_context(tc.tile_pool(name="expp", bufs=4))
    sbo = ctx.enter_context(tc.tile_pool(name="sbo", bufs=2))
    # PSUM: pse 3 x (128,1024) = 6 banks, pso 1 x (128,1024) = 2 banks
    pse = ctx.enter_context(tc.tile_pool(name="pse", bufs=3, space="PSUM"))
    pso = ctx.enter_context(tc.tile_pool(name="pso", bufs=1, space="PSUM"))

    Exp = mybir.ActivationFunctionType.Exp
    Mult = mybir.AluOpType.mult
    Add = mybir.AluOpType.add

    # PE p-state warm-up on memset garbage: the ramp completes during the
    # first input DMA waits instead of slowing the first energy matmuls
    ws = sbk.tile([64, 640], F32, tag="wsrc")
    nc.vector.memset(ws[:], 0.01)
    wps = pse.tile([128, 1024], F32, tag="e")
    for _ in range(5):
        nc.tensor.matmul(wps[:, 0:512], ws[:, 0:128].bitcast(F32R),
                         ws[:, 128:640].bitcast(F32R), start=True, stop=True)

    order = ORDER1 * reps
    n = len(order)

    def prologue(idx, chunked=False):
        s = order[idx]
        nk = SLOT_NK[s]
        mt = nk // 128
        xk = sbk.tile([64, nk], F32R, tag="xk")
        qag = sbq.tile([64, NQ], F32R, tag="qag")
        vt = sbv.tile([128, mt * 65], BF16, tag="vt")
        if chunked:
            # first energy tile's operands first; the bigger qag half leads
            # because SP dispatches serialize at ~565ns each, which adds
            # directly to the start of every later transfer
            nc.sync.dma_start(qag[:, 0:512], aps[f"qag{s}"][:, 0:512])
            nc.sync.dma_start(xk[:, 0:128], aps[f"xk{s}"][:, 0:128])
            nc.sync.dma_start(qag[:, 512:1024], aps[f"qag{s}"][:, 512:1024])
            off = 128
            while off < nk:
                w = min(512, nk - off)
                nc.sync.dma_start(xk[:, off:off + w],
                                  aps[f"xk{s}"][:, off:off + w])
                off += w
        else:
            nc.sync.dma_start(qag[:], aps[f"qag{s}"][:])
            nc.sync.dma_start(xk[:], aps[f"xk{s}"][:])
        nc.sync.dma_start(vt[:], aps[f"vt{s}"][:])
        vt3 = vt[:].rearrange("p (t c) -> p t c", c=65)
        return dict(s=s, nk=nk, mt=mt, xk=xk, qag=qag, vt3=vt3)

    tiles = []
    for idx in range(n):
        mt = SLOT_NK[order[idx]] // 128
        tiles += [(idx, t, mt) for t in range(mt)]
    NT = len(tiles)

    sts = {0: prologue(0, chunked=True)}
    if n > 1:
        sts[1] = prologue(1)

    def energy(g):
        idx, t, _ = tiles[g]
        st = sts[idx]
        eps = pse.tile([128, 1024], F32, tag="e")
        for o in (0, 512):
            nc.tensor.matmul(eps[:, o:o + 512],
                             st["xk"][:, 128 * t:128 * t + 128],
                             st["qag"][:, o:o + 512],
                             start=True, stop=True)
        return eps

    def expop(g, eps):
        if _sd_is_scalar(g):
            ex = expp.tile([128, 1024], BF16, tag="exs")
            nc.scalar.activation(ex[:], eps[:], Exp)
            return (ex, False)
        ex = expp.tile([128, 1024], I16, tag="exd")
        nc.vector.tensor_scalar(ex[:], eps[:], SCH_A, SCH_B, Mult, Add)
        return (ex, True)

    def attv(g, ops3, exinfo):
        idx, t, mt = tiles[g]
        st = sts[idx]
        ex, isd = exinfo
        for c in range(8):
            lhsT = ex[:, 128 * c:128 * c + 128]
            if isd:
                lhsT = lhsT.bitcast(BF16)
            co = (c % 4) * 65
            # start=True pending-zeroes the ENTIRE 2KB PSUM bank, so only
            # the first chunk of each bank may set it; the other chunks'
            # first write then lands on pending-zero bytes and overwrites
            # (zero+accumulate) correctly.
            nc.tensor.matmul(ops3[:, c // 4, co:co + 65], lhsT,
                             st["vt3"][:, t, :],
                             start=(t == 0 and c % 4 == 0),
                             stop=(t == mt - 1 and c % 4 == 3),
                             skip_group_check=True)

    def boundary(idx, ops, blocking_exp_scalar, last=False):
        """PSUM evacuation.  The next slot's first att@v waits on BOTH the
        next tile's exp (engine Y) and these copies, so the copies go on the
        OTHER engine Z to run concurrently with that exp.  Per-bank split
        releases the two pso banks' WARs independently."""
        st = sts[idx]
        fin = sbo.tile([128, 520], F32, tag="fin")
        src = ops[:].rearrange("p (g x) -> p g x", x=512)
        nc.scalar.copy(fin[:, 0:260], src[:, 0, 0:260])
        nc.sync.dma_start(aps["o"][st["s"]][:, 0:260], fin[:, 0:260])
        nc.vector.tensor_copy(fin[:, 260:520], src[:, 1, 0:260])
        nc.sync.dma_start(aps["o"][st["s"]][:, 260:520], fin[:, 260:520])

    # att@v lags one tile behind the emission front: the PE stream becomes
    # [E(g+2)ab, A(g-1)x8] so exp leads attv by 2 tiles (jitter absorption)
    # and a full energy pair sits between consecutive slots' attv groups,
    # covering the boundary-copy latency.
    epst = {0: energy(0)}
    if NT > 1:
        epst[1] = energy(1)
    exinfo = {0: expop(0, epst[0])}
    opst = {}
    for g in range(NT + 1):
        if g < NT:
            idx, t, mt = tiles[g]
            if t == 0 and idx + 2 < n:
                sts[idx + 2] = prologue(idx + 2)
            if g + 2 < NT:
                epst[g + 2] = energy(g + 2)
            if g + 1 < NT:
                exinfo[g + 1] = expop(g + 1, epst[g + 1])
            if t == 0:
                ops_t = pso.tile([128, 1024], F32, tag="o")
                opst[idx] = ops_t
        if g >= 1:
            pidx, pt, pmt = tiles[g - 1]
            ops3 = opst[pidx][:].rearrange("p (g x) -> p g x", x=512)
            attv(g - 1, ops3, exinfo.pop(g - 1))
            epst.pop(g - 1)
            if pt == pmt - 1:
                boundary(pidx, opst.pop(pidx), True, last=(g - 1 == NT - 1))
                sts.pop(pidx)


_CACHE = {}


def _build(reps):
    if reps in _CACHE:
        return _CACHE[reps]
    nc = bacc.Bacc("TRN2", target_bir_lowering=False, debug=False,
                   enable_asserts=True)
    aps = {}
    for s, nk in enumerate(SLOT_NK):
        aps[f"xk{s}"] = nc.dram_tensor(f"xk{s}", [64, nk], F32R,
                                       kind="ExternalInput").ap()
        aps[f"qag{s}"] = nc.dram_tensor(f"qag{s}", [64, NQ], F32R,
                                        kind="ExternalInput").ap()
        aps[f"vt{s}"] = nc.dram_tensor(f"vt{s}", [128, (nk // 128) * 65],
                                       BF16, kind="ExternalInput").ap()
    aps["o"] = nc.dram_tensor("o", [9, 128, 520], F32,
                              kind="ExternalOutput").ap()

    with tile.TileContext(nc) as tc:
        with ExitStack() as ctx:
            _emit(nc, tc, ctx, aps, reps)
    nc.compile()
    _CACHE[reps] = nc
    return nc


def _host_inputs(x, Wq, bq, Wk, bk, Wv, bv):
    x = np.asarray(x, np.float32)
    Wq = np.asarray(Wq, np.float32)
    Wk = np.asarray(Wk, np.float32)
    Wv = np.asarray(Wv, np.float32)
    bq = np.asarray(bq, np.float32)
    bv = np.asarray(bv, np.float32)

    xf = x.reshape(B, C, -1)
    Aq = Wk.T @ Wq                      # = (Wq^T Wk)^T
    cvec = Wk.T @ bq
    qag_full = (Aq @ xf + cvec[None, :, None]).reshape(B, C, H, W, T)
    v_full = (Wv @ xf + bv[None, :, None]).reshape(B, C, H, W, T)

    in_maps = []
    for c in range(N_CORES):
        m = {}
        for s, (b, i, j, z) in enumerate(TASKS[c]):
            nk_slot = SLOT_NK[s]
            mt = nk_slot // 128
            sx, dx = _win(i)
            sy, dy = _win(j)
            sz, dz = _win_z(z)
            nk = dx * dy * dz
            xkb = np.zeros((64, nk_slot), np.float32)
            xkb[:, :nk] = x[b, :, sx:sx + dx, sy:sy + dy,
                            sz:sz + dz].reshape(64, nk)
            m[f"xk{s}"] = xkb
            m[f"qag{s}"] = np.ascontiguousarray(
                qag_full[b, :, sx:sx + 16, sy:sy + 16,
                         sz:sz + 4].reshape(64, NQ))
            vp = np.zeros((65, nk_slot), np.float32)
            vp[:64, :nk] = v_full[b, :, sx:sx + dx, sy:sy + dy,
                                  sz:sz + dz].reshape(64, nk)
            vp[64, :nk] = 1.0
            m[f"vt{s}"] = np.ascontiguousarray(
                vp.reshape(65, mt, 128).transpose(2, 1, 0)
                .reshape(128, mt * 65)).astype(BF)
        in_maps.append(m)
    return in_maps


def _scatter(results, x):
    x = np.asarray(x, np.float32)
    out = np.empty((B, C, H, W, T), np.float32)
    for c in range(N_CORES):
        o = results[c]["o"]
        for s, (b, i, j, z) in enumerate(TASKS[c]):
            sx, _ = _win(i)
            sy, _ = _win(j)
            sz, _ = _win_z(z)
            fin3 = o[s].reshape(128, 2, 260)
            outT = np.empty((1024, 65), np.float32)
            for ch in range(8):
                outT[128 * ch:128 * ch + 128] = (
                    fin3[:, ch // 4, (ch % 4) * 65:(ch % 4) * 65 + 65])
            onrm = outT[:, :64] / outT[:, 64:65]
            blk = (onrm.T.reshape(64, 16, 16, 4) +
                   x[b, :, sx:sx + 16, sy:sy + 16, sz:sz + 4])
            out[b, :, sx:sx + 16, sy:sy + 16, sz:sz + 4] = blk
    return out


def _ensure_axon():
    # The axon PJRT plugin is registered by sitecustomize at interpreter
    # start; if a caller pinned JAX_PLATFORMS=cpu before jax init, try to
    # re-enable the axon backend (run_bass_via_pjrt needs 8 trn2 devices).
    import jax

    try:
        if any(d.platform == "axon" for d in jax.devices()):
            return
    except Exception:
        pass
    try:
        jax.config.update("jax_platforms", "axon,cpu")
        jax.extend.backend.clear_backends()
    except Exception:
        pass


def run(x, Wq, bq, Wk, bk, Wv, bv, reps=1):
    _ensure_axon()
    nc = _build(reps)
    in_maps = _host_inputs(x, Wq, bq, Wk, bk, Wv, bv)
    res = run_bass_kernel_spmd(nc, in_maps, core_ids=list(range(N_CORES)))
    return _scatter(res.results, x), res


def kernel(x, Wq, bq, Wk, bk, Wv, bv):
    out, _ = run(x, Wq, bq, Wk, bk, Wv, bv,
                 reps=int(os.environ.get("KREP", "1")))
    return out


# revision 40
# speedup vs baseline: 1.0420x; 1.0111x over previous
"""Trainium2 Bass kernel for nn_Block_self_attention_inter_intra_3D.

Math: the reference loops 36 overlapping windows (i,j in 0..2, z in 0..3) of a
(2,64,48,48,16) volume, runs channel-projected position attention inside each
window (reading the ORIGINAL x), and writes results back last-write-wins, so
each window "owns" exactly its local [0:16,0:16,0:4] sub-box (1024 queries)
attending to all N window key positions (N in {3456,2304,1536,1024}).

Device math per (window,batch) task, with everything foldable precomputed on
the host:
  energy[m,n] = qag[:,n] . xk[:,m]     qag = (Wk^T Wq) xq + Wk^T bq  (host),
                                       xk = raw window channels (64, NK).
                                       The k-bias term bk contributes a
                                       per-query constant => softmax-invariant
                                       => dropped.  K = 64 on the PE.
  w = exp(energy)                      split across TWO engines (see below)
  outT[n, 0:64] += w[m,n] * vT[m, :]   vT = (Wv x + bv)^T (host, bf16), with
  outT[n, 64]   += w[m,n]              an appended ones column for sum(exp).
  host: out = outT[:,:64]/outT[:,64:] + xq   (normalize + residual on host)

The att@v matmul runs TRANSPOSED (queries on PSUM partitions, 65 channels
free, bf16 operands): 8 x 65 = 520 PE rows per 128-key tile instead of 1024,
cutting PE time ~2x for that stage and making sum(exp) a PSUM column.

exp is the classic wall (ScalarE activation = 1 col/cycle @1.2GHz).  Tiles of
energies alternate between TWO engines (pattern 8:7 per 15 m-tiles):
  S: ScalarE true exp -> bf16
  D: DVE Schraudolph exp: bits16 = round(E*(2^7/ln2) + (127*2^7 - 7.75)) as a
     single fused tensor_scalar (mult+add, f32 PSUM -> int16 SBUF), bitcast
     to bf16.  HW-verified round-to-nearest conversion; max ~3% sawtooth
     error on those tiles only; end-to-end rel err ~3e-3 (tol 2e-2).
Both engines run concurrently on different PSUM energy tiles (3-deep ring of
(128,1024) tiles = 6 banks; att@v accumulator (128,1024) = 2 banks, chunks
packed 4-per-bank at 65-col offsets so no matmul crosses a bank).

Sharding: 72 (window,batch) tasks -> 8 cores x 9 uniform slots
(3x N=3456, 4x N=2304, 2x N=1536; the two N=1024 tasks land on core 7 padded
to 1536 with zeroed keys AND zeroed vT columns -- zero vT (incl. the ones
col) makes fake keys contribute exactly nothing, no mask needed).
"""

import os
import sys

sys.path.insert(0, "/opt/trn_rl_repo")

from contextlib import ExitStack

import numpy as np
import ml_dtypes

import concourse.bacc as bacc
import concourse.mybir as mybir
import concourse.tile as tile
from concourse.bass_utils import run_bass_kernel_spmd

F32 = mybir.dt.float32
F32R = mybir.dt.float32r
BF16 = mybir.dt.bfloat16
I16 = mybir.dt.int16
BF = ml_dtypes.bfloat16

N_CORES = 8
NQ = 1024
SLOT_NK = [3456, 3456, 3456, 2304, 2304, 2304, 2304, 1536, 1536]

# Schraudolph constants for bf16-bit exp on the DVE (round-to-nearest HW
# conversion, calibrated end-to-end: c = 7.75)
SCH_A = float(np.float32(128.0 / np.log(2.0)))
SCH_B = float(np.float32(127.0 * 128.0 - 7.75))

B, C, H, W, T = 2, 64, 48, 48, 16


def _win(i):
    s = 16 * i
    return s, min(s + 24, 48) - s


def _win_z(z):
    s = 4 * z
    return s, min(s + 6, 16) - s


def _task_lists():
    t3456 = [(b, i, j, z) for b in (0, 1) for i in (0, 1) for j in (0, 1)
             for z in (0, 1, 2)]
    t2304 = ([(b, i, j, 3) for b in (0, 1) for i in (0, 1) for j in (0, 1)] +
             [(b, i, 2, z) for b in (0, 1) for i in (0, 1) for z in (0, 1, 2)] +
             [(b, 2, j, z) for b in (0, 1) for j in (0, 1) for z in (0, 1, 2)])
    t1536 = ([(b, i, 2, 3) for b in (0, 1) for i in (0, 1)] +
             [(b, 2, j, 3) for b in (0, 1) for j in (0, 1)] +
             [(b, 2, 2, z) for b in (0, 1) for z in (0, 1, 2)])
    t1024 = [(b, 2, 2, 3) for b in (0, 1)]
    assert len(t3456) == 24 and len(t2304) == 32
    assert len(t1536) == 14 and len(t1024) == 2
    tail = t1536 + t1024
    per_core = []
    for c in range(N_CORES):
        per_core.append(t3456[3 * c:3 * c + 3] + t2304[4 * c:4 * c + 4] +
                        tail[2 * c:2 * c + 2])
    return per_core


TASKS = _task_lists()

# smallest slot first so the pipeline fills during the first (small) DMAs
ORDER1 = [7, 0, 3, 1, 4, 2, 5, 6, 8]


def _sd_is_scalar(g):
    """Exp engine for global m-tile g: 8 ScalarE / 7 DVE per 15 tiles."""
    return (g % 15) % 2 == 0


def _emit(nc, tc, ctx, aps, reps):
    sbk = ctx.enter_context(tc.tile_pool(name="sbk", bufs=3))
    sbq = ctx.enter_context(tc.tile_pool(name="sbq", bufs=3))
    sbv = ctx.enter_context(tc.tile_pool(name="sbv", bufs=3))
    expp = ctx.enter_context(tc.tile_pool(name="expp", bufs=4))
    sbo = ctx.enter_context(tc.tile_pool(name="sbo", bufs=2))
    # PSUM: pse 3 x (128,1024) = 6 banks, pso 1 x (128,1024) = 2 banks
    pse = ctx.enter_context(tc.tile_pool(name="pse", bufs=3, space="PSUM"))
    pso = ctx.enter_context(tc.tile_pool(name="pso", bufs=1, space="PSUM"))

    Exp = mybir.ActivationFunctionType.Exp
    Mult = mybir.AluOpType.mult
    Add = mybir.AluOpType.add

    # PE p-state warm-up on memset garbage: the ramp completes during the
    # first input DMA waits instead of slowing the first energy matmuls
    ws = sbk.tile([64, 640], F32, tag="wsrc")
    nc.vector.memset(ws[:], 0.01)
    wps = pse.tile([128, 1024], F32, tag="e")
    for _ in range(5):
        nc.tensor.matmul(wps[:, 0:512], ws[:, 0:128].bitcast(F32R),
                         ws[:, 128:640].bitcast(F32R), start=True, stop=True)

    order = ORDER1 * reps
    n = len(order)

    def prologue(idx, chunked=False):
        s = order[idx]
        nk = SLOT_NK[s]
        mt = nk // 128
        xk = sbk.tile([64, nk], F32R, tag="xk")
        qag = sbq.tile([64, NQ], F32R, tag="qag")
        vt = sbv.tile([128, mt * 65], BF16, tag="vt")
        if chunked:
            # first energy tile's operands first; the bigger qag half leads
            # because SP dispatches serialize at ~565ns each, which adds
            # directly to the start of every later transfer
            nc.sync.dma_start(qag[:, 0:512], aps[f"qag{s}"][:, 0:512])
            nc.sync.dma_start(xk[:, 0:128], aps[f"xk{s}"][:, 0:128])
            nc.sync.dma_start(qag[:, 512:1024], aps[f"qag{s}"][:, 512:1024])
            off = 128
            while off < nk:
                w = min(512, nk - off)
                nc.sync.dma_start(xk[:, off:off + w],
                                  aps[f"xk{s}"][:, off:off + w])
                off += w
        else:
            nc.sync.dma_start(qag[:], aps[f"qag{s}"][:])
            nc.sync.dma_start(xk[:], aps[f"xk{s}"][:])
        nc.sync.dma_start(vt[:], aps[f"vt{s}"][:])
        vt3 = vt[:].rearrange("p (t c) -> p t c", c=65)
        return dict(s=s, nk=nk, mt=mt, xk=xk, qag=qag, vt3=vt3)

    tiles = []
    for idx in range(n):
        mt = SLOT_NK[order[idx]] // 128
        tiles += [(idx, t, mt) for t in range(mt)]
    NT = len(tiles)

    # S/D exp assignment: strict alternation, with the ratio-balancing
    # extra S tiles pinned to slot starts where the boundary slack absorbs
    # the double-S seam (mid-slot seams beat against the slot-pair period
    # and stall the PE ring).
    sd_list = []
    alt = True
    for (idx_, t_, mt_) in tiles:
        if t_ == 0 and len(sd_list) > 0:
            sd_list.append(True)
            alt = False
        else:
            sd_list.append(alt)
            alt = not alt

    sts = {0: prologue(0, chunked=True)}
    if n > 1:
        sts[1] = prologue(1)

    def energy(g):
        idx, t, _ = tiles[g]
        st = sts[idx]
        eps = pse.tile([128, 1024], F32, tag="e")
        for o in (0, 512):
            nc.tensor.matmul(eps[:, o:o + 512],
                             st["xk"][:, 128 * t:128 * t + 128],
                             st["qag"][:, o:o + 512],
                             start=True, stop=True)
        return eps

    def expop(g, eps):
        if sd_list[g]:
            ex = expp.tile([128, 1024], BF16, tag="exs")
            nc.scalar.activation(ex[:], eps[:], Exp)
            return (ex, False)
        ex = expp.tile([128, 1024], I16, tag="exd")
        nc.vector.tensor_scalar(ex[:], eps[:], SCH_A, SCH_B, Mult, Add)
        return (ex, True)

    def attv(g, ops3, exinfo):
        idx, t, mt = tiles[g]
        st = sts[idx]
        ex, isd = exinfo
        for c in range(8):
            lhsT = ex[:, 128 * c:128 * c + 128]
            if isd:
                lhsT = lhsT.bitcast(BF16)
            co = (c % 4) * 65
            # start=True pending-zeroes the ENTIRE 2KB PSUM bank, so only
            # the first chunk of each bank may set it; the other chunks'
            # first write then lands on pending-zero bytes and overwrites
            # (zero+accumulate) correctly.
            nc.tensor.matmul(ops3[:, c // 4, co:co + 65], lhsT,
                             st["vt3"][:, t, :],
                             start=(t == 0 and c % 4 == 0),
                             stop=(t == mt - 1 and c % 4 == 3),
                             skip_group_check=True)

    def boundary(idx, ops, blocking_exp_scalar, last=False):
        """PSUM evacuation.  The next slot's first att@v waits on BOTH the
        next tile's exp (engine Y) and these copies, so the copies go on the
        OTHER engine Z to run concurrently with that exp.  Per-bank split
        releases the two pso banks' WARs independently."""
        st = sts[idx]
        fin = sbo.tile([128, 520], F32, tag="fin")
        src = ops[:].rearrange("p (g x) -> p g x", x=512)
        nc.scalar.copy(fin[:, 0:260], src[:, 0, 0:260])
        nc.sync.dma_start(aps["o"][st["s"]][:, 0:260], fin[:, 0:260])
        nc.vector.tensor_copy(fin[:, 260:520], src[:, 1, 0:260])
        nc.sync.dma_start(aps["o"][st["s"]][:, 260:520], fin[:, 260:520])

    # att@v lags one tile behind the emission front: the PE stream becomes
    # [E(g+2)ab, A(g-1)x8] so exp leads attv by 2 tiles (jitter absorption)
    # and a full energy pair sits between consecutive slots' attv groups,
    # covering the boundary-copy latency.
    epst = {0: energy(0)}
    if NT > 1:
        epst[1] = energy(1)
    exinfo = {0: expop(0, epst[0])}
    opst = {}
    for g in range(NT + 1):
        if g < NT:
            idx, t, mt = tiles[g]
            if t == 0 and idx + 2 < n:
                sts[idx + 2] = prologue(idx + 2)
            if g + 2 < NT:
                epst[g + 2] = energy(g + 2)
            if g + 1 < NT:
                exinfo[g + 1] = expop(g + 1, epst[g + 1])
            if t == 0:
                ops_t = pso.tile([128, 1024], F32, tag="o")
                opst[idx] = ops_t
        if g >= 1:
            pidx, pt, pmt = tiles[g - 1]
            ops3 = opst[pidx][:].rearrange("p (g x) -> p g x", x=512)
            attv(g - 1, ops3, exinfo.pop(g - 1))
            epst.pop(g - 1)
            if pt == pmt - 1:
                boundary(pidx, opst.pop(pidx), True, last=(g - 1 == NT - 1))
                sts.pop(pidx)


_CACHE = {}


def _build(reps):
    if reps in _CACHE:
        return _CACHE[reps]
    nc = bacc.Bacc("TRN2", target_bir_lowering=False, debug=False,
                   enable_asserts=True)
    aps = {}
    for s, nk in enumerate(SLOT_NK):
        aps[f"xk{s}"] = nc.dram_tensor(f"xk{s}", [64, nk], F32R,
                                       kind="ExternalInput").ap()
        aps[f"qag{s}"] = nc.dram_tensor(f"qag{s}", [64, NQ], F32R,
                                        kind="ExternalInput").ap()
        aps[f"vt{s}"] = nc.dram_tensor(f"vt{s}", [128, (nk // 128) * 65],
                                       BF16, kind="ExternalInput").ap()
    aps["o"] = nc.dram_tensor("o", [9, 128, 520], F32,
                              kind="ExternalOutput").ap()

    with tile.TileContext(nc) as tc:
        with ExitStack() as ctx:
            _emit(nc, tc, ctx, aps, reps)
    nc.compile()
    _CACHE[reps] = nc
    return nc


def _host_inputs(x, Wq, bq, Wk, bk, Wv, bv):
    x = np.asarray(x, np.float32)
    Wq = np.asarray(Wq, np.float32)
    Wk = np.asarray(Wk, np.float32)
    Wv = np.asarray(Wv, np.float32)
    bq = np.asarray(bq, np.float32)
    bv = np.asarray(bv, np.float32)

    xf = x.reshape(B, C, -1)
    Aq = Wk.T @ Wq                      # = (Wq^T Wk)^T
    cvec = Wk.T @ bq
    qag_full = (Aq @ xf + cvec[None, :, None]).reshape(B, C, H, W, T)
    v_full = (Wv @ xf + bv[None, :, None]).reshape(B, C, H, W, T)

    in_maps = []
    for c in range(N_CORES):
        m = {}
        for s, (b, i, j, z) in enumerate(TASKS[c]):
            nk_slot = SLOT_NK[s]
            mt = nk_slot // 128
            sx, dx = _win(i)
            sy, dy = _win(j)
            sz, dz = _win_z(z)
            nk = dx * dy * dz
            xkb = np.zeros((64, nk_slot), np.float32)
            xkb[:, :nk] = x[b, :, sx:sx + dx, sy:sy + dy,
                            sz:sz + dz].reshape(64, nk)
            m[f"xk{s}"] = xkb
            m[f"qag{s}"] = np.ascontiguousarray(
                qag_full[b, :, sx:sx + 16, sy:sy + 16,
                         sz:sz + 4].reshape(64, NQ))
            vp = np.zeros((65, nk_slot), np.float32)
            vp[:64, :nk] = v_full[b, :, sx:sx + dx, sy:sy + dy,
                                  sz:sz + dz].reshape(64, nk)
            vp[64, :nk] = 1.0
            m[f"vt{s}"] = np.ascontiguousarray(
                vp.reshape(65, mt, 128).transpose(2, 1, 0)
                .reshape(128, mt * 65)).astype(BF)
        in_maps.append(m)
    return in_maps


def _scatter(results, x):
    x = np.asarray(x, np.float32)
    out = np.empty((B, C, H, W, T), np.float32)
    for c in range(N_CORES):
        o = results[c]["o"]
        for s, (b, i, j, z) in enumerate(TASKS[c]):
            sx, _ = _win(i)
            sy, _ = _win(j)
            sz, _ = _win_z(z)
            fin3 = o[s].reshape(128, 2, 260)
            outT = np.empty((1024, 65), np.float32)
            for ch in range(8):
                outT[128 * ch:128 * ch + 128] = (
                    fin3[:, ch // 4, (ch % 4) * 65:(ch % 4) * 65 + 65])
            onrm = outT[:, :64] / outT[:, 64:65]
            blk = (onrm.T.reshape(64, 16, 16, 4) +
                   x[b, :, sx:sx + 16, sy:sy + 16, sz:sz + 4])
            out[b, :, sx:sx + 16, sy:sy + 16, sz:sz + 4] = blk
    return out


def _ensure_axon():
    # The axon PJRT plugin is registered by sitecustomize at interpreter
    # start; if a caller pinned JAX_PLATFORMS=cpu before jax init, try to
    # re-enable the axon backend (run_bass_via_pjrt needs 8 trn2 devices).
    import jax

    try:
        if any(d.platform == "axon" for d in jax.devices()):
            return
    except Exception:
        pass
    try:
        jax.config.update("jax_platforms", "axon,cpu")
        jax.extend.backend.clear_backends()
    except Exception:
        pass


def run(x, Wq, bq, Wk, bk, Wv, bv, reps=1):
    _ensure_axon()
    nc = _build(reps)
    in_maps = _host_inputs(x, Wq, bq, Wk, bk, Wv, bv)
    res = run_bass_kernel_spmd(nc, in_maps, core_ids=list(range(N_CORES)))
    return _scatter(res.results, x), res


def kernel(x, Wq, bq, Wk, bk, Wv, bv):
    out, _ = run(x, Wq, bq, Wk, bk, Wv, bv,
                 reps=int(os.environ.get("KREP", "1")))
    return out
